# revision 35
# baseline (speedup 1.0000x reference)
"""MultiHeadAttention TRN2 Bass kernel (8 NeuronCores), fp8 DoubleRow version.

Sharding: core c = (batch b = c//2, query-half = c%2). Each core computes
K/V for its full batch (2048 keys) and attention + output projection + LN
for its 1024 query rows. No collectives; host gathers per-core outputs.

Device math (fp8e4m3 matmuls in DoubleRow perf mode = 2 k-tiles of 128
contracted per instruction at 0.5 cycles per output column):
  Q/K proj: out [hd(2 heads x 64dk), m]  = w8[d,2,hd].T @ qt8[d,2,m]
  V proj:   out [m, 2 heads x 64]        = qt8[d,2,m].T @ wv8[d,2,hv]
  kht2/qht2: DMA re-layout to [32, 2(dk-half), m] per head so S can run
  DoubleRow with dk = 2x32.
  S[m, q]   = kht2[32,2,mtile].T @ qht2[32,2,q]      (per head, per m-tile)
  E = exp(S/32): q-columns 0:512 on ACT (Exp), 512:1024 on DVE via a
      custom op (0.5*(1+x/8)^2+0.5)^8 ~ e^x, <1% typical error. The two
      S psum halves (spa/spb) are separate pools so each engine's
      pipeline is self-paced.
  O[q, 64]  = sum_p E[m,2,q].T @ V[m,2,64]           (E is the stationary)
  den[q]    = sum_p E[m,2,q].T @ ones[m,2,1]
  CT[q, (h,64)] = O * (1/den)    (per-partition scalar broadcast)
  ctT = DMA-transpose(CT bf16) -> fp8;  Y = ctT.T @ pw8 + residual; LayerNorm
"""
import numpy as np
import ml_dtypes

import concourse.bass as bass
import concourse.mybir as mybir
import concourse.tile as tile
from concourse import bacc
from concourse.bass_utils import run_bass_kernel_spmd

# ---------------- custom DVE exp op ----------------
import concourse.dve_ops as dve_ops
from concourse.dve_spec import Spec, Src0, C0, C1, C2, sq


def _exp_ref(in0, in1, c0, c1, c2):
    x = in0.astype(np.float32)
    t = ((x * c0 + c1) ** 2 * c2 + c2).astype(np.float32)
    t = (t * t).astype(np.float32)
    t = (t * t).astype(np.float32)
    return (t * t).astype(np.float32)


def _get_exp_op():
    name = "EXP_APPROX_ANT"
    for op in dve_ops.OPS:
        if op.name == name:
            return op
    body = sq(sq(sq(sq(Src0 * C0 + C1) * C2 + C2)))
    op = dve_ops.DveOp(
        name,
        Spec(body=body, reference=_exp_ref),
        subdim=False,
        uops_sha={"v3": "7cc04e385f99d2ac", "v4": "4c6dc6b0499997cd"},
    )
    row = max(dve_ops._SUB_OPCODE_FOR_NAME.values()) + 1
    assert row < 0x20
    dve_ops.OPS.append(op)
    dve_ops.CUSTOM_DVE_SPECS[name] = op.spec
    dve_ops._SUB_OPCODE_FOR_NAME[name] = row
    return op


EXP_OP = _get_exp_op()


def _addred_ref(in0, in1, c0, c1, c2):
    b = (in0.astype(np.float32) + in1.astype(np.float32)).astype(np.float32)
    return b, b.reshape(b.shape[0], -1).sum(axis=-1, keepdims=True)


def _get_addred_op():
    from operator import add as _add
    from concourse.dve_spec import Src1, Zero
    name = "ADD_REDUCE_ANT"
    for op in dve_ops.OPS:
        if op.name == name:
            return op
    op = dve_ops.DveOp(
        name,
        Spec(body=Src0 + Src1, accum=_add, accum_init=Zero,
             reference=_addred_ref),
        subdim=False,
        uops_sha={"v3": "8be32207425579a6", "v4": "102f3739dc9078fe"},
    )
    row = max(dve_ops._SUB_OPCODE_FOR_NAME.values()) + 1
    assert row < 0x20
    dve_ops.OPS.append(op)
    dve_ops.CUSTOM_DVE_SPECS[name] = op.spec
    dve_ops._SUB_OPCODE_FOR_NAME[name] = row
    return op


ADDRED_OP = _get_addred_op()

F32 = mybir.dt.float32
BF16 = mybir.dt.bfloat16
F8 = mybir.dt.float8e4
AF = mybir.ActivationFunctionType
ALU = mybir.AluOpType
AX = mybir.AxisListType
PM = mybir.MatmulPerfMode

B, L, D = 4, 2048, 1024
H, DK = 16, 64
HALF = 1024
TEMPER = 32.0
LN_EPS = 1e-3
NP8 = ml_dtypes.float8_e4m3

_CACHE = {}


def build(iters=1):
    nc = bacc.Bacc(None, target_bir_lowering=False)
    qt8_d = nc.dram_tensor("qt8", [128, 8, L], F8, kind="ExternalInput")
    wq8_d = nc.dram_tensor("wq8", [128, 8, H * DK], F8, kind="ExternalInput")
    wk8_d = nc.dram_tensor("wk8", [128, 8, H * DK], F8, kind="ExternalInput")
    wv8_d = nc.dram_tensor("wv8", [128, 8, H * DK], F8, kind="ExternalInput")
    pw8_d = nc.dram_tensor("pw8", [128, 8, D], F8, kind="ExternalInput")
    ones_d = nc.dram_tensor("ones8", [1, 2], F8, kind="ExternalInput")
    qres_d = nc.dram_tensor("qres", [HALF, D], BF16, kind="ExternalInput")
    lna_d = nc.dram_tensor("lna", [1, D], F32, kind="ExternalInput")
    lnb_d = nc.dram_tensor("lnb", [1, D], F32, kind="ExternalInput")
    out_d = nc.dram_tensor("out", [HALF, D], F32, kind="ExternalOutput")

    with tile.TileContext(nc) as tc:
        with (
            tc.tile_pool(name="c1", bufs=1) as c1,
            tc.tile_pool(name="st", bufs=2) as st,
            tc.tile_pool(name="eh", bufs=2) as ehp,
            tc.tile_pool(name="sm", bufs=2) as smp,
            tc.tile_pool(name="psQ", bufs=2, space="PSUM") as psQ,
            tc.tile_pool(name="psD", bufs=1, space="PSUM") as psD,
            tc.tile_pool(name="psSA", bufs=2, space="PSUM") as psSA,
            tc.tile_pool(name="psSB", bufs=2, space="PSUM") as psSB,
            tc.tile_pool(name="psO", bufs=1, space="PSUM") as psO,
        ):
            # persistent SBUF
            qt8 = c1.tile([128, 8, L], F8, name="qt8_t")
            wq8 = c1.tile([128, 8, H * DK], F8, name="wq8_t")
            wk8 = c1.tile([128, 8, H * DK], F8, name="wk8_t")
            wv8 = c1.tile([128, 8, H * DK], F8, name="wv8_t")
            pw8 = c1.tile([128, 8, D], F8, name="pw8_t")
            ones8 = c1.tile([128, 2, 1], F8, name="ones8_t")
            kht2 = c1.tile([128, 6, 2, L], F8, name="kht2_t")
            qht2 = c1.tile([128, 6, 2, HALF], F8, name="qht2_t")
            vaug = c1.tile([128, 16, H * DK], F8, name="vaug_t")
            ctTb = c1.tile([128, 8, D], BF16, name="ctTb_t")
            ctT8 = c1.tile([128, 8, D], F8, name="ctT8_t")
            lna_t = c1.tile([128, D], F32, name="lna_t")
            lnb_t = c1.tile([128, D], F32, name="lnb_t")

            nc.sync.dma_start(wk8[:], wk8_d[:])
            nc.sync.dma_start(qt8[:, :, 0:512], qt8_d[:, :, 0:512])
            nc.sync.dma_start(wq8[:], wq8_d[:])
            for mc4 in range(1, 4):
                nc.sync.dma_start(qt8[:, :, mc4 * 512:(mc4 + 1) * 512],
                                  qt8_d[:, :, mc4 * 512:(mc4 + 1) * 512])
            nc.sync.dma_start(wv8[:], wv8_d[:])
            nc.sync.dma_start(ones8[:, :, 0], ones_d[:].to_broadcast([128, 2]))

            for it in range(iters):
                P = f"it{it}_"

                evac_n = [0]

                def evac(dst, src):
                    """PSUM->SBUF fp8 conversion. First 26 (lead-in, both
                    engines idle) alternate DVE/ACT; the rest on ACT (DVE
                    carries exp+div in steady state)."""
                    evac_n[0] += 1
                    if (evac_n[0] <= 26 and evac_n[0] % 2 == 0) or \
                            (evac_n[0] > 26 and evac_n[0] % 5 == 0):
                        nc.vector.tensor_copy(dst, src)
                    else:
                        nc.scalar.activation(dst, src, AF.Copy)

                def qk_proj(hp, w8, ncols, dst2, name):
                    """Project K (ncols=L) or Q (ncols=HALF) for head-pair hp,
                    evacuate as fp8, then DMA re-layout into dst2's
                    [32, 2(dk-half), m] per-head layout."""
                    chunk = st.tile([128, ncols], F8, name=f"{P}{name}c_{hp}",
                                    tag=f"{name}stage")
                    for mc in range(ncols // 512):
                        ps = psQ.tile([128, 512], F32,
                                      name=f"{P}{name}ps_{hp}_{mc}", tag="ps")
                        for cc in range(2):
                            for j in range(4):
                                nc.tensor.matmul(
                                    ps[:, cc * 256:(cc + 1) * 256],
                                    w8[:, 2 * j:2 * j + 2,
                                       hp * 128:(hp + 1) * 128],
                                    qt8[:, 2 * j:2 * j + 2,
                                        mc * 512 + cc * 256:
                                        mc * 512 + (cc + 1) * 256],
                                    start=(j == 0), stop=(j == 3),
                                    perf_mode=PM.DoubleRow,
                                )
                        evac(chunk[:, mc * 512:(mc + 1) * 512], ps[:])
                    for h2 in range(2):
                        h = 2 * hp + h2
                        for t in range(2):
                            nc.sync.dma_start(
                                dst2[32 * (h % 3):32 * (h % 3) + 32,
                                     h // 3, t, :],
                                chunk[64 * h2 + 32 * t:64 * h2 + 32 * t + 32, :],
                            )

                def v_proj(mt, vhalf):
                    """V for m-tile mt, heads [8*vhalf, 8*vhalf+8)."""
                    ps = psQ.tile([128, 512], F32, name=f"{P}vps_{mt}_{vhalf}",
                                  tag="ps")
                    for g in range(4):
                        hp = vhalf * 4 + g
                        for j in range(4):
                            nc.tensor.matmul(
                                ps[:, g * 128:(g + 1) * 128],
                                qt8[:, 2 * j:2 * j + 2,
                                    mt * 128:(mt + 1) * 128],
                                wv8[:, 2 * j:2 * j + 2,
                                    hp * 128:(hp + 1) * 128],
                                start=(j == 0), stop=(j == 3),
                                perf_mode=PM.DoubleRow,
                            )
                    evac(vaug[:, mt, vhalf * 512:(vhalf + 1) * 512], ps[:])

                ehs = {}
                ots = {}
                dens = {}
                recbs = {}

                def attn_ov(h):
                    """O + den matmuls for head h (PE), emitted one head late
                    so the division pipeline never blocks the engines."""
                    eh, ot, den = ehs[h], ots[h], dens[h]
                    for qt in range(8):
                        for p in range(8):
                            nc.tensor.matmul(
                                ot[:, qt, :],
                                eh[:, p, :, qt * 128:(qt + 1) * 128],
                                vaug[:, 2 * p:2 * p + 2, h * 64:(h + 1) * 64],
                                start=(p == 0), stop=(p == 7),
                                perf_mode=PM.DoubleRow,
                            )
                            # same stationary as the ot matmul above -> the
                            # PE can skip the weight reload
                            nc.tensor.matmul(
                                den[:, qt:qt + 1],
                                eh[:, p, :, qt * 128:(qt + 1) * 128],
                                ones8[:, :, 0:1],
                                start=(p == 0), stop=(p == 7),
                                perf_mode=PM.DoubleRow,
                            )

                def attn_recb(h):
                    recb = smp.tile([128, 8], F32, name=f"{P}rec_{h}",
                                    tag="recb")
                    recbs[h] = recb
                    nc.vector.reciprocal_approx_fast(recb[:], dens[h][:])

                cbq = [None]

                def attn_div(h):
                    """CT = O * (1/den) in ONE stride-0-broadcast DVE op per
                    head, into a 4-head staging block; transpose per quad."""
                    ot, recb = ots[h], recbs[h]
                    if h % 4 == 0:
                        cbq[0] = st.tile([128, 8, 4, 64], BF16,
                                         name=f"{P}cbq_{h // 4}",
                                         tag="ctblk", bufs=2)
                    cb = cbq[0]
                    nc.vector.tensor_tensor(
                        cb[:, :, h % 4, :], ot[:, :, :],
                        recb[:].to_broadcast([128, 8, 64]), ALU.mult)
                    if h % 4 == 3:
                        for qt in range(8):
                            nc.sync.dma_start_transpose(
                                ctTb[:, 2 * (h // 4):2 * (h // 4) + 2,
                                     qt * 128:(qt + 1) * 128],
                                cb[:, qt, :, :])

                def attn_sx(h):
                    """S matmuls + exp for head h; interleaves the previous
                    head's O/den/recb/div at stall-free points."""
                    a, b2 = h % 3, h // 3
                    eh = ehp.tile([128, 8, 2, HALF], F8, name=f"{P}eh_{h}",
                                  tag="eh")
                    ehs[h] = eh
                    ots[h] = psO.tile([128, 8, 64], F32, name=f"{P}ot_{h}",
                                      tag="ot")
                    dens[h] = psD.tile([128, 8], F32, name=f"{P}den_{h}",
                                       tag="den")
                    for mt in range(16):
                        spa = psSA.tile([128, 512], F32,
                                        name=f"{P}spa_{h}_{mt}", tag="sa")
                        spb = psSB.tile([128, 512], F32,
                                        name=f"{P}spb_{h}_{mt}", tag="sb")
                        for qc in range(4):
                            sp = spa if qc < 2 else spb
                            nc.tensor.matmul(
                                sp[:, (qc % 2) * 256:(qc % 2 + 1) * 256],
                                kht2[32 * a:32 * a + 32, b2, :,
                                     mt * 128:(mt + 1) * 128],
                                qht2[32 * a:32 * a + 32, b2, :,
                                     qc * 256:(qc + 1) * 256],
                                start=True, stop=True,
                                perf_mode=PM.DoubleRow,
                            )
                        nc.scalar.activation(
                            eh[:, mt // 2, mt % 2, 0:512], spa[:],
                            AF.Exp, scale=1.0 / TEMPER)
                        nc.vector._custom_dve(
                            EXP_OP, out=eh[:, mt // 2, mt % 2, 512:HALF],
                            in0=spb[:],
                            s0=1.0 / (TEMPER * 8.0), s1=1.0, imm2=0.5)
                        if mt == 3 and h > 0:
                            attn_ov(h - 1)
                        if mt == 5 and h > 0:
                            attn_recb(h - 1)
                    if h > 0:
                        attn_div(h - 1)

                def attn_last(h):
                    attn_ov(h)
                    attn_recb(h)
                    attn_div(h)

                # ---- emission schedule: overlap projections with attention --
                qk_proj(0, wq8, HALF, qht2, "q")
                qk_proj(0, wk8, L, kht2, "k")
                attn_sx(0)
                qk_proj(1, wq8, HALF, qht2, "q")
                qk_proj(1, wk8, L, kht2, "k")
                for mt in range(16):
                    v_proj(mt, 0)
                attn_sx(1)
                qrs = {}

                def load_qr(qt):
                    qr = st.tile([128, D], BF16, name=f"{P}qr_{qt}",
                                 tag="qr", bufs=6)
                    qrs[qt] = qr
                    nc.sync.dma_start(qr[:], qres_d[qt * 128:(qt + 1) * 128, :])

                for h in range(2, 16):
                    if 2 <= h <= 7:
                        qk_proj(h, wq8, HALF, qht2, "q")
                        qk_proj(h, wk8, L, kht2, "k")
                    if h == 13:
                        for qt in range(6):
                            load_qr(qt)


                    if h == 6:
                        for mt in range(16):
                            v_proj(mt, 1)
                        nc.sync.dma_start(pw8[:], pw8_d[:])
                        nc.sync.dma_start(lna_t[:],
                                          lna_d[:].to_broadcast([128, D]))
                        nc.sync.dma_start(lnb_t[:],
                                          lnb_d[:].to_broadcast([128, D]))
                    attn_sx(h)
                attn_last(15)

                # ---- convert ctTb -> fp8 ------------------------------------
                for qt in range(8):
                    if qt % 2 == 0:
                        nc.vector.tensor_copy(
                            ctT8[:, :, qt * 128:(qt + 1) * 128],
                            ctTb[:, :, qt * 128:(qt + 1) * 128])
                    else:
                        nc.scalar.activation(
                            ctT8[:, :, qt * 128:(qt + 1) * 128],
                            ctTb[:, :, qt * 128:(qt + 1) * 128], AF.Copy)
                for qt in range(8):
                    if qt >= 6:
                        load_qr(qt)
                    qr = qrs[qt]
                    yt = st.tile([128, D], F32, name=f"{P}yt_{qt}", tag="yt")
                    sh = smp.tile([128, 2], F32, name=f"{P}sh_{qt}", tag="ln_sh")
                    for oc in range(2):
                        yp = psQ.tile([128, 512], F32,
                                      name=f"{P}yp_{qt}_{oc}", tag="ps")
                        for cc in range(2):
                            for j in range(4):
                                nc.tensor.matmul(
                                    yp[:, cc * 256:(cc + 1) * 256],
                                    ctT8[:, 2 * j:2 * j + 2,
                                         qt * 128:(qt + 1) * 128],
                                    pw8[:, 2 * j:2 * j + 2,
                                        oc * 512 + cc * 256:
                                        oc * 512 + (cc + 1) * 256],
                                    start=(j == 0), stop=(j == 3),
                                    perf_mode=PM.DoubleRow,
                                )
                        # fused residual add + row-sum accumulate
                        nc.vector._custom_dve(
                            ADDRED_OP, out=yt[:, oc * 512:(oc + 1) * 512],
                            in0=yp[:], in1=qr[:, oc * 512:(oc + 1) * 512],
                            accum_out=sh[:, oc:oc + 1])
                    # layernorm: mu, sigma (ddof=1), (y-mu)/(sigma+eps)*a+b
                    o_t = st.tile([128, D], F32, name=f"{P}o_{qt}", tag="o")
                    s = smp.tile([128, 1], F32, name=f"{P}s_{qt}", tag="ln_s")
                    nc.vector.tensor_add(s[:], sh[:, 0:1], sh[:, 1:2])
                    negmean = smp.tile([128, 1], F32, name=f"{P}nm_{qt}",
                                       tag="ln_nm")
                    nc.vector.tensor_scalar_mul(negmean[:], s[:], -1.0 / D)
                    mean = smp.tile([128, 1], F32, name=f"{P}m_{qt}", tag="ln_m")
                    nc.vector.tensor_scalar_mul(mean[:], s[:], 1.0 / D)
                    ss = smp.tile([128, 1], F32, name=f"{P}ss_{qt}", tag="ln_ss")
                    nc.scalar.activation(o_t[:], yt[:], AF.Square,
                                         bias=negmean[:], accum_out=ss[:])
                    sigma = smp.tile([128, 1], F32, name=f"{P}sg_{qt}",
                                     tag="ln_sg")
                    nc.scalar.activation(sigma[:], ss[:], AF.Sqrt,
                                         scale=1.0 / (D - 1))
                    dd = smp.tile([128, 1], F32, name=f"{P}dd_{qt}", tag="ln_dd")
                    nc.vector.tensor_scalar_add(dd[:], sigma[:], LN_EPS)
                    rec2 = smp.tile([128, 1], F32, name=f"{P}rc_{qt}",
                                    tag="ln_rc")
                    nc.vector.reciprocal_approx_fast(rec2[:], dd[:])
                    nc.vector.tensor_scalar(o_t[:], yt[:], mean[:], rec2[:],
                                            ALU.subtract, ALU.mult)
                    nc.gpsimd.tensor_mul(o_t[:], o_t[:], lna_t[:])
                    nc.gpsimd.tensor_add(o_t[:], o_t[:], lnb_t[:])
                    nc.sync.dma_start(out_d[qt * 128:(qt + 1) * 128, :], o_t[:])

    nc.compile()
    return nc


def _get_nc():
    if "nc" not in _CACHE:
        _CACHE["nc"] = build()
    return _CACHE["nc"]


def _interleave8(a):
    """[D, N] f32 -> [128, 8, N] fp8 with [p, 2j+t, n] = a[j*256+t*128+p, n]."""
    n = a.shape[1]
    return np.ascontiguousarray(
        a.reshape(4, 2, 128, n).transpose(2, 0, 1, 3).reshape(128, 8, n)
    ).astype(NP8)


def _in_maps(q, w_qs, w_ks, w_vs, proj_w, proj_b, ln_a, ln_b):
    wq8 = _interleave8(np.ascontiguousarray(
        w_qs.transpose(1, 0, 2).reshape(D, H * DK)))
    wk8 = _interleave8(np.ascontiguousarray(
        w_ks.transpose(1, 0, 2).reshape(D, H * DK)))
    wv8 = _interleave8(np.ascontiguousarray(
        w_vs.transpose(1, 0, 2).reshape(D, H * DK)))
    pw8 = _interleave8(np.ascontiguousarray(proj_w.T))
    ones8 = np.ones((1, 2), NP8)
    lna = np.ascontiguousarray(ln_a[None, :]).astype(np.float32)
    lnb = np.ascontiguousarray(ln_b[None, :]).astype(np.float32)
    maps = []
    for c in range(8):
        b, half = c // 2, c % 2
        qb = q[b]
        perm = np.r_[half * HALF:(half + 1) * HALF,
                     (1 - half) * HALF:(2 - half) * HALF]
        qt8 = _interleave8(np.ascontiguousarray(qb.T[:, perm]))
        qres = np.ascontiguousarray(
            qb[half * HALF:(half + 1) * HALF, :] + proj_b[None, :]
        ).astype(ml_dtypes.bfloat16)
        maps.append({
            "qt8": qt8, "qres": qres,
            "wq8": wq8, "wk8": wk8, "wv8": wv8, "pw8": pw8,
            "ones8": ones8, "lna": lna, "lnb": lnb,
        })
    return maps


def kernel(q, w_qs, w_ks, w_vs, proj_w, proj_b, ln_a, ln_b, **kw):
    q = np.asarray(q, dtype=np.float32)
    w_qs = np.asarray(w_qs, dtype=np.float32)
    w_ks = np.asarray(w_ks, dtype=np.float32)
    w_vs = np.asarray(w_vs, dtype=np.float32)
    proj_w = np.asarray(proj_w, dtype=np.float32)
    proj_b = np.asarray(proj_b, dtype=np.float32)
    ln_a = np.asarray(ln_a, dtype=np.float32)
    ln_b = np.asarray(ln_b, dtype=np.float32)

    in_maps = _in_maps(q, w_qs, w_ks, w_vs, proj_w, proj_b, ln_a, ln_b)
    nc = _get_nc()
    res = run_bass_kernel_spmd(nc, in_maps, core_ids=list(range(8))).results

    out = np.empty((B, L, D), dtype=np.float32)
    for c in range(8):
        b, half = c // 2, c % 2
        out[b, half * HALF:(half + 1) * HALF, :] = res[c]["out"]
    return out


# revision 36
# speedup vs baseline: 1.0145x; 1.0145x over previous
"""MultiHeadAttention TRN2 Bass kernel (8 NeuronCores), fp8 DoubleRow version.

Sharding: core c = (batch b = c//2, query-half = c%2). Each core computes
K/V for its full batch (2048 keys) and attention + output projection + LN
for its 1024 query rows. No collectives; host gathers per-core outputs.

Device math (fp8e4m3 matmuls in DoubleRow perf mode = 2 k-tiles of 128
contracted per instruction at 0.5 cycles per output column):
  Q/K proj: out [hd(2 heads x 64dk), m]  = w8[d,2,hd].T @ qt8[d,2,m]
  V proj:   out [m, 2 heads x 64]        = qt8[d,2,m].T @ wv8[d,2,hv]
  kht2/qht2: DMA re-layout to [32, 2(dk-half), m] per head so S can run
  DoubleRow with dk = 2x32.
  S[m, q]   = kht2[32,2,mtile].T @ qht2[32,2,q]      (per head, per m-tile)
  E = exp(S/32): q-columns 0:512 on ACT (Exp), 512:1024 on DVE via a
      custom op (0.5*(1+x/8)^2+0.5)^8 ~ e^x, <1% typical error. The two
      S psum halves (spa/spb) are separate pools so each engine's
      pipeline is self-paced.
  O[q, 64]  = sum_p E[m,2,q].T @ V[m,2,64]           (E is the stationary)
  den[q]    = sum_p E[m,2,q].T @ ones[m,2,1]
  CT[q, (h,64)] = O * (1/den)    (per-partition scalar broadcast)
  ctT = DMA-transpose(CT bf16) -> fp8;  Y = ctT.T @ pw8 + residual; LayerNorm
"""
import numpy as np
import ml_dtypes

import concourse.bass as bass
import concourse.mybir as mybir
import concourse.tile as tile
from concourse import bacc
from concourse.bass_utils import run_bass_kernel_spmd

# ---------------- custom DVE exp op ----------------
import concourse.dve_ops as dve_ops
from concourse.dve_spec import Spec, Src0, C0, C1, C2, sq


def _exp_ref(in0, in1, c0, c1, c2):
    x = in0.astype(np.float32)
    t = ((x * c0 + c1) ** 2 * c2 + c2).astype(np.float32)
    t = (t * t).astype(np.float32)
    t = (t * t).astype(np.float32)
    return (t * t).astype(np.float32)


def _get_exp_op():
    name = "EXP_APPROX_ANT"
    for op in dve_ops.OPS:
        if op.name == name:
            return op
    body = sq(sq(sq(sq(Src0 * C0 + C1) * C2 + C2)))
    op = dve_ops.DveOp(
        name,
        Spec(body=body, reference=_exp_ref),
        subdim=False,
        uops_sha={"v3": "7cc04e385f99d2ac", "v4": "4c6dc6b0499997cd"},
    )
    row = max(dve_ops._SUB_OPCODE_FOR_NAME.values()) + 1
    assert row < 0x20
    dve_ops.OPS.append(op)
    dve_ops.CUSTOM_DVE_SPECS[name] = op.spec
    dve_ops._SUB_OPCODE_FOR_NAME[name] = row
    return op


EXP_OP = _get_exp_op()


def _addred_ref(in0, in1, c0, c1, c2):
    b = (in0.astype(np.float32) + in1.astype(np.float32)).astype(np.float32)
    return b, b.reshape(b.shape[0], -1).sum(axis=-1, keepdims=True)


def _get_addred_op():
    from operator import add as _add
    from concourse.dve_spec import Src1, Zero
    name = "ADD_REDUCE_ANT"
    for op in dve_ops.OPS:
        if op.name == name:
            return op
    op = dve_ops.DveOp(
        name,
        Spec(body=Src0 + Src1, accum=_add, accum_init=Zero,
             reference=_addred_ref),
        subdim=False,
        uops_sha={"v3": "8be32207425579a6", "v4": "102f3739dc9078fe"},
    )
    row = max(dve_ops._SUB_OPCODE_FOR_NAME.values()) + 1
    assert row < 0x20
    dve_ops.OPS.append(op)
    dve_ops.CUSTOM_DVE_SPECS[name] = op.spec
    dve_ops._SUB_OPCODE_FOR_NAME[name] = row
    return op


ADDRED_OP = _get_addred_op()

F32 = mybir.dt.float32
BF16 = mybir.dt.bfloat16
F8 = mybir.dt.float8e4
AF = mybir.ActivationFunctionType
ALU = mybir.AluOpType
AX = mybir.AxisListType
PM = mybir.MatmulPerfMode

B, L, D = 4, 2048, 1024
H, DK = 16, 64
HALF = 1024
TEMPER = 32.0
LN_EPS = 1e-3
NP8 = ml_dtypes.float8_e4m3

_CACHE = {}


def build(iters=1):
    nc = bacc.Bacc(None, target_bir_lowering=False)
    qt8_d = nc.dram_tensor("qt8", [128, 8, L], F8, kind="ExternalInput")
    wq8_d = nc.dram_tensor("wq8", [128, 8, H * DK], F8, kind="ExternalInput")
    wk8_d = nc.dram_tensor("wk8", [128, 8, H * DK], F8, kind="ExternalInput")
    wv8_d = nc.dram_tensor("wv8", [128, 8, H * DK], F8, kind="ExternalInput")
    pw8_d = nc.dram_tensor("pw8", [128, 8, D], F8, kind="ExternalInput")
    ones_d = nc.dram_tensor("ones8", [1, 2], F8, kind="ExternalInput")
    qres_d = nc.dram_tensor("qres", [HALF, D], BF16, kind="ExternalInput")
    lna_d = nc.dram_tensor("lna", [1, D], F32, kind="ExternalInput")
    lnb_d = nc.dram_tensor("lnb", [1, D], F32, kind="ExternalInput")
    out_d = nc.dram_tensor("out", [HALF, D], F32, kind="ExternalOutput")

    with tile.TileContext(nc) as tc:
        with (
            tc.tile_pool(name="c1", bufs=1) as c1,
            tc.tile_pool(name="st", bufs=2) as st,
            tc.tile_pool(name="eh", bufs=2) as ehp,
            tc.tile_pool(name="sm", bufs=2) as smp,
            tc.tile_pool(name="psQ", bufs=2, space="PSUM") as psQ,
            tc.tile_pool(name="psD", bufs=1, space="PSUM") as psD,
            tc.tile_pool(name="psSA", bufs=2, space="PSUM") as psSA,
            tc.tile_pool(name="psSB", bufs=2, space="PSUM") as psSB,
            tc.tile_pool(name="psO", bufs=1, space="PSUM") as psO,
        ):
            # persistent SBUF
            qt8 = c1.tile([128, 8, L], F8, name="qt8_t")
            wq8 = c1.tile([128, 8, H * DK], F8, name="wq8_t")
            wk8 = c1.tile([128, 8, H * DK], F8, name="wk8_t")
            wv8 = c1.tile([128, 8, H * DK], F8, name="wv8_t")
            pw8 = c1.tile([128, 8, D], F8, name="pw8_t")
            ones8 = c1.tile([128, 2, 1], F8, name="ones8_t")
            kht2 = c1.tile([128, 6, 2, L], F8, name="kht2_t")
            qht2 = c1.tile([128, 6, 2, HALF], F8, name="qht2_t")
            vaug = c1.tile([128, 16, H * DK], F8, name="vaug_t")
            ctTb = c1.tile([128, 8, D], BF16, name="ctTb_t")
            ctT8 = c1.tile([128, 8, D], F8, name="ctT8_t")
            lna_t = c1.tile([128, D], F32, name="lna_t")
            lnb_t = c1.tile([128, D], F32, name="lnb_t")

            nc.sync.dma_start(wk8[:], wk8_d[:])
            nc.sync.dma_start(qt8[:, :, 0:512], qt8_d[:, :, 0:512])
            nc.sync.dma_start(wq8[:], wq8_d[:])
            for mc4 in range(1, 4):
                nc.sync.dma_start(qt8[:, :, mc4 * 512:(mc4 + 1) * 512],
                                  qt8_d[:, :, mc4 * 512:(mc4 + 1) * 512])
            nc.sync.dma_start(wv8[:], wv8_d[:])
            nc.sync.dma_start(ones8[:, :, 0], ones_d[:].to_broadcast([128, 2]))

            for it in range(iters):
                P = f"it{it}_"

                evac_n = [0]

                def evac(dst, src):
                    """PSUM->SBUF fp8 conversion. First 26 (lead-in, both
                    engines idle) alternate DVE/ACT; the rest on ACT (DVE
                    carries exp+div in steady state)."""
                    evac_n[0] += 1
                    if (evac_n[0] <= 26 and evac_n[0] % 2 == 0) or \
                            (evac_n[0] > 26 and evac_n[0] % 5 == 0):
                        nc.vector.tensor_copy(dst, src)
                    else:
                        nc.scalar.activation(dst, src, AF.Copy)

                def qk_proj(hp, w8, ncols, dst2, name):
                    """Project K (ncols=L) or Q (ncols=HALF) for head-pair hp,
                    evacuate as fp8, then DMA re-layout into dst2's
                    [32, 2(dk-half), m] per-head layout."""
                    chunk = st.tile([128, ncols], F8, name=f"{P}{name}c_{hp}",
                                    tag=f"{name}stage")
                    for mc in range(ncols // 512):
                        ps = psQ.tile([128, 512], F32,
                                      name=f"{P}{name}ps_{hp}_{mc}", tag="ps")
                        for cc in range(2):
                            for j in range(4):
                                nc.tensor.matmul(
                                    ps[:, cc * 256:(cc + 1) * 256],
                                    w8[:, 2 * j:2 * j + 2,
                                       hp * 128:(hp + 1) * 128],
                                    qt8[:, 2 * j:2 * j + 2,
                                        mc * 512 + cc * 256:
                                        mc * 512 + (cc + 1) * 256],
                                    start=(j == 0), stop=(j == 3),
                                    perf_mode=PM.DoubleRow,
                                )
                        evac(chunk[:, mc * 512:(mc + 1) * 512], ps[:])
                    for h2 in range(2):
                        h = 2 * hp + h2
                        for t in range(2):
                            nc.sync.dma_start(
                                dst2[32 * (h % 3):32 * (h % 3) + 32,
                                     h // 3, t, :],
                                chunk[64 * h2 + 32 * t:64 * h2 + 32 * t + 32, :],
                            )

                def v_proj(mt, vhalf):
                    """V for m-tile mt, heads [8*vhalf, 8*vhalf+8)."""
                    ps = psQ.tile([128, 512], F32, name=f"{P}vps_{mt}_{vhalf}",
                                  tag="ps")
                    for g in range(4):
                        hp = vhalf * 4 + g
                        for j in range(4):
                            nc.tensor.matmul(
                                ps[:, g * 128:(g + 1) * 128],
                                qt8[:, 2 * j:2 * j + 2,
                                    mt * 128:(mt + 1) * 128],
                                wv8[:, 2 * j:2 * j + 2,
                                    hp * 128:(hp + 1) * 128],
                                start=(j == 0), stop=(j == 3),
                                perf_mode=PM.DoubleRow,
                            )
                    evac(vaug[:, mt, vhalf * 512:(vhalf + 1) * 512], ps[:])

                ehs = {}
                ots = {}
                dens = {}
                recbs = {}

                def attn_ov(h):
                    """O + den matmuls for head h (PE), emitted one head late
                    so the division pipeline never blocks the engines."""
                    eh, ot, den = ehs[h], ots[h], dens[h]
                    for qt in range(8):
                        for p in range(8):
                            nc.tensor.matmul(
                                ot[:, qt, :],
                                eh[:, p, :, qt * 128:(qt + 1) * 128],
                                vaug[:, 2 * p:2 * p + 2, h * 64:(h + 1) * 64],
                                start=(p == 0), stop=(p == 7),
                                perf_mode=PM.DoubleRow,
                            )
                            # same stationary as the ot matmul above -> the
                            # PE can skip the weight reload
                            nc.tensor.matmul(
                                den[:, qt:qt + 1],
                                eh[:, p, :, qt * 128:(qt + 1) * 128],
                                ones8[:, :, 0:1],
                                start=(p == 0), stop=(p == 7),
                                perf_mode=PM.DoubleRow,
                            )

                def attn_recb(h):
                    recb = smp.tile([128, 8], F32, name=f"{P}rec_{h}",
                                    tag="recb")
                    recbs[h] = recb
                    nc.vector.reciprocal_approx_fast(recb[:], dens[h][:])

                cbq = [None]

                def attn_div(h):
                    """CT = O * (1/den) in ONE stride-0-broadcast DVE op per
                    head, into a 4-head staging block; transpose per quad."""
                    ot, recb = ots[h], recbs[h]
                    if h % 4 == 0:
                        cbq[0] = st.tile([128, 8, 4, 64], BF16,
                                         name=f"{P}cbq_{h // 4}",
                                         tag="ctblk", bufs=2)
                    cb = cbq[0]
                    nc.vector.tensor_tensor(
                        cb[:, :, h % 4, :], ot[:, :, :],
                        recb[:].to_broadcast([128, 8, 64]), ALU.mult)
                    if h % 4 == 3:
                        for qt in range(8):
                            nc.sync.dma_start_transpose(
                                ctTb[:, 2 * (h // 4):2 * (h // 4) + 2,
                                     qt * 128:(qt + 1) * 128],
                                cb[:, qt, :, :])

                def attn_sx(h):
                    """S matmuls + exp for head h; interleaves the previous
                    head's O/den/recb/div at stall-free points."""
                    a, b2 = h % 3, h // 3
                    eh = ehp.tile([128, 8, 2, HALF], F8, name=f"{P}eh_{h}",
                                  tag="eh")
                    ehs[h] = eh
                    ots[h] = psO.tile([128, 8, 64], F32, name=f"{P}ot_{h}",
                                      tag="ot")
                    dens[h] = psD.tile([128, 8], F32, name=f"{P}den_{h}",
                                       tag="den")
                    for mt in range(16):
                        spa = psSA.tile([128, 512], F32,
                                        name=f"{P}spa_{h}_{mt}", tag="sa")
                        spb = psSB.tile([128, 512], F32,
                                        name=f"{P}spb_{h}_{mt}", tag="sb")
                        for qc in range(4):
                            sp = spa if qc < 2 else spb
                            nc.tensor.matmul(
                                sp[:, (qc % 2) * 256:(qc % 2 + 1) * 256],
                                kht2[32 * a:32 * a + 32, b2, :,
                                     mt * 128:(mt + 1) * 128],
                                qht2[32 * a:32 * a + 32, b2, :,
                                     qc * 256:(qc + 1) * 256],
                                start=True, stop=True,
                                perf_mode=PM.DoubleRow,
                            )
                        nc.scalar.activation(
                            eh[:, mt // 2, mt % 2, 0:512], spa[:],
                            AF.Exp, scale=1.0 / TEMPER)
                        nc.vector._custom_dve(
                            EXP_OP, out=eh[:, mt // 2, mt % 2, 512:HALF],
                            in0=spb[:],
                            s0=1.0 / (TEMPER * 8.0), s1=1.0, imm2=0.5)
                        if mt == 3 and h > 0:
                            attn_ov(h - 1)
                        if mt == 5 and h > 0:
                            attn_recb(h - 1)
                    if h > 0:
                        attn_div(h - 1)

                def attn_last(h):
                    attn_ov(h)
                    attn_recb(h)
                    attn_div(h)

                # ---- emission schedule: overlap projections with attention --
                qk_proj(0, wq8, HALF, qht2, "q")
                qk_proj(0, wk8, L, kht2, "k")
                attn_sx(0)
                qk_proj(1, wq8, HALF, qht2, "q")
                qk_proj(1, wk8, L, kht2, "k")
                for mt in range(16):
                    v_proj(mt, 0)
                attn_sx(1)
                qrs = {}

                def load_qr(qt):
                    qr = st.tile([128, D], BF16, name=f"{P}qr_{qt}",
                                 tag="qr", bufs=6)
                    qrs[qt] = qr
                    nc.sync.dma_start(qr[:], qres_d[qt * 128:(qt + 1) * 128, :])

                for h in range(2, 16):
                    if 2 <= h <= 7:
                        qk_proj(h, wq8, HALF, qht2, "q")
                        qk_proj(h, wk8, L, kht2, "k")
                    if h == 13:
                        for qt in range(6):
                            load_qr(qt)


                    if h == 6:
                        for mt in range(16):
                            v_proj(mt, 1)
                        nc.sync.dma_start(pw8[:], pw8_d[:])
                        nc.sync.dma_start(lna_t[:],
                                          lna_d[:].to_broadcast([128, D]))
                        nc.sync.dma_start(lnb_t[:],
                                          lnb_d[:].to_broadcast([128, D]))
                    attn_sx(h)
                attn_last(15)

                # ---- convert ctTb -> fp8 ------------------------------------
                for qt in range(8):
                    nc.gpsimd.tensor_copy(
                        ctT8[:, :, qt * 128:(qt + 1) * 128],
                        ctTb[:, :, qt * 128:(qt + 1) * 128])
                for qt in range(8):
                    if qt >= 6:
                        load_qr(qt)
                    qr = qrs[qt]
                    yt = st.tile([128, D], F32, name=f"{P}yt_{qt}", tag="yt")
                    sh = smp.tile([128, 2], F32, name=f"{P}sh_{qt}", tag="ln_sh")
                    for oc in range(2):
                        yp = psQ.tile([128, 512], F32,
                                      name=f"{P}yp_{qt}_{oc}", tag="ps")
                        for cc in range(2):
                            for j in range(4):
                                nc.tensor.matmul(
                                    yp[:, cc * 256:(cc + 1) * 256],
                                    ctT8[:, 2 * j:2 * j + 2,
                                         qt * 128:(qt + 1) * 128],
                                    pw8[:, 2 * j:2 * j + 2,
                                        oc * 512 + cc * 256:
                                        oc * 512 + (cc + 1) * 256],
                                    start=(j == 0), stop=(j == 3),
                                    perf_mode=PM.DoubleRow,
                                )
                        # fused residual add + row-sum accumulate
                        nc.vector._custom_dve(
                            ADDRED_OP, out=yt[:, oc * 512:(oc + 1) * 512],
                            in0=yp[:], in1=qr[:, oc * 512:(oc + 1) * 512],
                            accum_out=sh[:, oc:oc + 1])
                    # layernorm: mu, sigma (ddof=1), (y-mu)/(sigma+eps)*a+b
                    o_t = st.tile([128, D], F32, name=f"{P}o_{qt}", tag="o")
                    s = smp.tile([128, 1], F32, name=f"{P}s_{qt}", tag="ln_s")
                    nc.vector.tensor_add(s[:], sh[:, 0:1], sh[:, 1:2])
                    negmean = smp.tile([128, 1], F32, name=f"{P}nm_{qt}",
                                       tag="ln_nm")
                    nc.vector.tensor_scalar_mul(negmean[:], s[:], -1.0 / D)
                    mean = smp.tile([128, 1], F32, name=f"{P}m_{qt}", tag="ln_m")
                    nc.vector.tensor_scalar_mul(mean[:], s[:], 1.0 / D)
                    ss = smp.tile([128, 1], F32, name=f"{P}ss_{qt}", tag="ln_ss")
                    nc.scalar.activation(o_t[:], yt[:], AF.Square,
                                         bias=negmean[:], accum_out=ss[:])
                    sigma = smp.tile([128, 1], F32, name=f"{P}sg_{qt}",
                                     tag="ln_sg")
                    nc.scalar.activation(sigma[:], ss[:], AF.Sqrt,
                                         scale=1.0 / (D - 1))
                    dd = smp.tile([128, 1], F32, name=f"{P}dd_{qt}", tag="ln_dd")
                    nc.vector.tensor_scalar_add(dd[:], sigma[:], LN_EPS)
                    rec2 = smp.tile([128, 1], F32, name=f"{P}rc_{qt}",
                                    tag="ln_rc")
                    nc.vector.reciprocal_approx_fast(rec2[:], dd[:])
                    nc.vector.tensor_scalar(o_t[:], yt[:], mean[:], rec2[:],
                                            ALU.subtract, ALU.mult)
                    nc.gpsimd.tensor_mul(o_t[:], o_t[:], lna_t[:])
                    nc.gpsimd.tensor_add(o_t[:], o_t[:], lnb_t[:])
                    nc.sync.dma_start(out_d[qt * 128:(qt + 1) * 128, :], o_t[:])

    nc.compile()
    return nc


def _get_nc():
    if "nc" not in _CACHE:
        _CACHE["nc"] = build()
    return _CACHE["nc"]


def _interleave8(a):
    """[D, N] f32 -> [128, 8, N] fp8 with [p, 2j+t, n] = a[j*256+t*128+p, n]."""
    n = a.shape[1]
    return np.ascontiguousarray(
        a.reshape(4, 2, 128, n).transpose(2, 0, 1, 3).reshape(128, 8, n)
    ).astype(NP8)


def _in_maps(q, w_qs, w_ks, w_vs, proj_w, proj_b, ln_a, ln_b):
    wq8 = _interleave8(np.ascontiguousarray(
        w_qs.transpose(1, 0, 2).reshape(D, H * DK)))
    wk8 = _interleave8(np.ascontiguousarray(
        w_ks.transpose(1, 0, 2).reshape(D, H * DK)))
    wv8 = _interleave8(np.ascontiguousarray(
        w_vs.transpose(1, 0, 2).reshape(D, H * DK)))
    pw8 = _interleave8(np.ascontiguousarray(proj_w.T))
    ones8 = np.ones((1, 2), NP8)
    lna = np.ascontiguousarray(ln_a[None, :]).astype(np.float32)
    lnb = np.ascontiguousarray(ln_b[None, :]).astype(np.float32)
    maps = []
    for c in range(8):
        b, half = c // 2, c % 2
        qb = q[b]
        perm = np.r_[half * HALF:(half + 1) * HALF,
                     (1 - half) * HALF:(2 - half) * HALF]
        qt8 = _interleave8(np.ascontiguousarray(qb.T[:, perm]))
        qres = np.ascontiguousarray(
            qb[half * HALF:(half + 1) * HALF, :] + proj_b[None, :]
        ).astype(ml_dtypes.bfloat16)
        maps.append({
            "qt8": qt8, "qres": qres,
            "wq8": wq8, "wk8": wk8, "wv8": wv8, "pw8": pw8,
            "ones8": ones8, "lna": lna, "lnb": lnb,
        })
    return maps


def kernel(q, w_qs, w_ks, w_vs, proj_w, proj_b, ln_a, ln_b, **kw):
    q = np.asarray(q, dtype=np.float32)
    w_qs = np.asarray(w_qs, dtype=np.float32)
    w_ks = np.asarray(w_ks, dtype=np.float32)
    w_vs = np.asarray(w_vs, dtype=np.float32)
    proj_w = np.asarray(proj_w, dtype=np.float32)
    proj_b = np.asarray(proj_b, dtype=np.float32)
    ln_a = np.asarray(ln_a, dtype=np.float32)
    ln_b = np.asarray(ln_b, dtype=np.float32)

    in_maps = _in_maps(q, w_qs, w_ks, w_vs, proj_w, proj_b, ln_a, ln_b)
    nc = _get_nc()
    res = run_bass_kernel_spmd(nc, in_maps, core_ids=list(range(8))).results

    out = np.empty((B, L, D), dtype=np.float32)
    for c in range(8):
        b, half = c // 2, c % 2
        out[b, half * HALF:(half + 1) * HALF, :] = res[c]["out"]
    return out
